# revision 93
# baseline (speedup 1.0000x reference)
"""Causal self-attention (B=4, S=2048, D=1024, H=16) on 8 TRN2 NeuronCores.

Sharding (tensor-parallel on heads + data-parallel on batch):
  core c -> batch c//2, head-half c%2 (8 of 16 heads).
  Wq/Wk/Wv column-split, Wo row-split; the two partial outputs per batch are
  summed on the host (+ bo), which is the row-parallel unshard.

Per-core Bass/Tile program (matmul operands bf16, psum/softmax fp32):
  phase A: qT/kT feature-major projections into 2-bank psum tiles, drained
           alternately by ACT/DVE; q is written as two zero-padded copies
           (qT0 rows 64:128 = 0, qT1 rows 0:64 = 0) so phase-B scores
           matmuls use full-width [128,128] kT stationaries (FWL-eligible)
           with K=128 zero-padded moving operands.
  phase B: per head-pair / 512-query superblock / 128-key tile:
           scoresT for both heads into one [128,1024] 2-bank psum tile (kT
           full stationary, qT0/qT1 moving), ONE merged exp per key tile,
           causal mask applied post-exp as a 0/1 multiply on probs (off the
           scores->exp chain), PV with full [128,128] v_aug stationaries
           whose ones-columns (cols 0:64, all heads) replicate sumexp into
           psum rows 0:64 for free; normalization is reciprocal_approx_fast
           (rows 0:64 only -- the custom op breaks at nonzero base
           partition) + DMA partition-shift + multiply.  PV emission is
           software-pipelined one key tile behind scores to hide the exp
           latency; v projection (next superblock) and output projection
           (previous superblock) are interleaved per head-pair as PE filler
           for the exp-latency bubbles.
  phase C tail: last superblock's output projection, with two tiles
           pre-accumulating p4=0..2 in spare psum before the final
           normalize lands.
"""

from contextlib import ExitStack

import numpy as np
import ml_dtypes

import concourse.bass as bass
import concourse.bacc as bacc
import concourse.tile as tile
import concourse.mybir as mybir

F32 = mybir.dt.float32
F32R = mybir.dt.float32r
BF16 = mybir.dt.bfloat16
def r(ap):
    return ap.bitcast(F32R)


def build_core_program(S=2048, D=1024, HC=8, DH=64, SQ=512, mm_dt=BF16,
                       qk_psum_bufs=2, probs_bufs=4):
    """Build the per-core Bass program (SPMD: same program, different data).
    mm_dt: dtype of matmul operands (BF16 or F32R). When BF16, the host must
    pass xT/wqk/wv/wo as bfloat16 arrays."""
    DQ = HC * DH              # head-slice width (512)
    DK = D // 128             # contraction tiles for projections (8)
    DQN = DQ // 128           # head-pair tiles (4)
    NSB = S // SQ             # query superblocks (4)
    NTT = S // 128            # token tiles (16)
    NOUT = min(512, D)        # output-proj free width
    NOB = D // NOUT           # output-proj col blocks (2)
    assert DQ % 128 == 0 and S % SQ == 0 and SQ % 128 == 0 and D % 128 == 0
    assert (S // SQ) % 2 == 0

    bf = mm_dt == BF16
    in_dt = BF16 if bf else F32

    def m(ap):
        # bitcast for f32->f32r reinterpretation; no-op for bf16 tiles
        return ap if bf else ap.bitcast(F32R)

    nc = bacc.Bacc("TRN2", target_bir_lowering=False, debug=False)

    xT = nc.dram_tensor("xT", [D, S], in_dt, kind="ExternalInput").ap()
    wqk = nc.dram_tensor("wqk", [D, 2 * DQ], in_dt, kind="ExternalInput").ap()
    wv = nc.dram_tensor("wv", [D, DQ], in_dt, kind="ExternalInput").ap()
    wo = nc.dram_tensor("wo", [DQ, D], in_dt, kind="ExternalInput").ap()
    bqk = nc.dram_tensor("bqk", [2 * DQ], F32, kind="ExternalInput").ap()
    bv = nc.dram_tensor("bv", [DQ], F32, kind="ExternalInput").ap()
    out = nc.dram_tensor("out", [S, D], F32, kind="ExternalOutput").ap()

    with tile.TileContext(nc) as tc, ExitStack() as ctx:
        ctx.enter_context(nc.allow_low_precision(
            reason="low-precision matmul operands; accumulation stays fp32"))
        const = ctx.enter_context(tc.tile_pool(name="const", bufs=1))
        big = ctx.enter_context(tc.tile_pool(name="big", bufs=1))
        stream = ctx.enter_context(tc.tile_pool(name="stream", bufs=1))
        psum = ctx.enter_context(tc.tile_pool(name="psum", bufs=1, space="PSUM"))

        # ---- constants ----
        # biases: bqk as [128, 2*DQN] (column t = dout tile t), bv broadcast.
        # Issued from gpsimd so the sync queue starts on the wqk/x loads.
        bqk_sb = const.tile([128, 2 * DQN], F32)
        nc.gpsimd.dma_start(bqk_sb[:], bqk.rearrange("(t p) -> p t", p=128))
        bv_rowf = const.tile([1, DQ], F32)
        nc.gpsimd.dma_start(bv_rowf[:], bv.rearrange("(a d) -> a d", a=1))


        # ---- big resident tensors ----
        kT = big.tile([128, DQN, S], mm_dt)     # [pair 2x64 rows, tokens]
        qT0 = big.tile([128, DQN, S], mm_dt)    # head0 q in rows 0:64, rest 0
        qT1 = big.tile([128, DQN, S], mm_dt)    # head1 q in rows 64:128, rest 0
        # v_aug layout [p, tt, hh, hp, 128]: ones in cols 0:64, v feats in
        # cols 64:128 for every head.  During PV the ones columns replicate
        # sumexp into psum rows 0:64 (where reciprocal_approx_fast works;
        # it breaks at non-zero base partitions) and attn lands at 64:128.
        v_aug = big.tile([128, NTT, 2, DQN, 128], mm_dt)
        wqk_sb = big.tile([128, DK, 2 * DQ], mm_dt)
        wv_sb = big.tile([128, DK, DQ], mm_dt)
        wo_sb = big.tile([128, DQN, D], mm_dt)
        xt_all = big.tile([128, DK, S], mm_dt)

        # wqk/xt first, interleaved in consumption (kt) order; issue cost is
        # ~0.6us per dma_start, so split across two idle engine queues
        for kt in range(DK):
            eng = nc.sync if kt % 2 == 0 else nc.scalar
            eng.dma_start(wqk_sb[:, kt, :],
                          m(wqk[128 * kt:128 * (kt + 1), :]))
            eng.dma_start(xt_all[:, kt, :], m(xT[128 * kt:128 * (kt + 1), :]))
        # causal mask [128,128]: 1 where p <= f else 0, multiplied into probs
        # AFTER exp (keeps the mask off the scores->exp critical chain)
        tri01 = const.tile([128, 128], mm_dt)
        nc.vector.memset(tri01[:], 1.0)
        nc.gpsimd.affine_select(
            out=tri01[:], in_=tri01[:], compare_op=mybir.AluOpType.is_ge,
            fill=0.0, base=0, channel_multiplier=-1, pattern=[[1, 128]],
        )
        nc.gpsimd.memset(qT0[64:128, :, :], 0.0)   # gpsimd is mostly idle
        nc.vector.memset(qT1[0:64, :, :], 0.0)

        # ===== phase A-qk: projection chain (2x2-bank psum tiles) ==========
        def emit_qk_chain(dt):
            pss = [psum.tile([128, 2 * SQ], F32, tag="qk", bufs=qk_psum_bufs,
                             name=f"pss_{dt}_{pr}") for pr in range(NSB // 2)]
            for kt in range(DK):
                for tb in range(NSB):
                    nc.tensor.matmul(
                        pss[tb // 2][:, (tb % 2) * SQ:(tb % 2 + 1) * SQ],
                        m(wqk_sb[:, kt, 128 * dt:128 * (dt + 1)]),
                        m(xt_all[:, kt, tb * SQ:(tb + 1) * SQ]),
                        start=(kt == 0), stop=(kt == DK - 1))
            is_q = dt < DQN
            hp = dt % DQN
            for pr in range(NSB // 2):
                cols = slice(2 * SQ * pr, 2 * SQ * (pr + 1))
                on_act = pr % 2 == 0  # alternate ACT/DVE to halve drain time
                if is_q:
                    for dest, rows in ((qT0, slice(0, 64)),
                                       (qT1, slice(64, 128))):
                        if on_act:
                            nc.scalar.activation(
                                dest[rows, hp, cols], pss[pr][rows, :],
                                mybir.ActivationFunctionType.Identity,
                                bias=bqk_sb[rows, dt:dt + 1], scale=0.125)
                        else:
                            nc.vector.tensor_scalar(
                                dest[rows, hp, cols], pss[pr][rows, :],
                                0.125, bqk_sb[rows, dt:dt + 1],
                                op0=mybir.AluOpType.mult,
                                op1=mybir.AluOpType.add)
                else:
                    if on_act:
                        nc.scalar.activation(
                            kT[:, hp, cols], pss[pr][:],
                            mybir.ActivationFunctionType.Identity,
                            bias=bqk_sb[:, dt:dt + 1], scale=1.0)
                    else:
                        nc.vector.tensor_scalar(
                            kT[:, hp, cols], pss[pr][:],
                            bqk_sb[:, dt:dt + 1], None,
                            op0=mybir.AluOpType.add)

        # only hp=0's chains run up front; chains for hp+1 are interleaved
        # at superblock 0's hp boundaries (PE filler while sb0's attention
        # would otherwise wait on the ACT exp chain)
        emit_qk_chain(0)
        emit_qk_chain(DQN)

        def emit_v_tile(tt):
            # v projection for one 128-token tile (token-stationary)
            psv = psum.tile([128, DQ], F32, tag="out", bufs=2,
                            name=f"psv_{tt}")
            for kt in range(DK):
                nc.tensor.matmul(
                    psv[:], m(xt_all[:, kt, 128 * tt:128 * (tt + 1)]),
                    m(wv_sb[:, kt, :]),
                    start=(kt == 0), stop=(kt == DK - 1))
            # psv col = h*64 + c with h = 2*hp + hh -> [p, hh, hp, c]
            psv5 = psv[:].rearrange("p (q hh c) -> p hh q c", hh=2, c=DH)
            bv5 = bv_bc[:].rearrange("p (q hh c) -> p hh q c", hh=2, c=DH)
            nc.vector.tensor_tensor(
                v_aug[:, tt, :, :, DH:128], psv5[:], bv5[:],
                op=mybir.AluOpType.add)

        attn_tiles = {}

        def emit_out_tile(tt):
            # output projection for one 128-token tile of a done superblock
            atp = attn_tiles[tt // (SQ // 128)]
            mm_ = tt % (SQ // 128)
            pos = [psum.tile([128, NOUT], F32, tag="out", bufs=2,
                             name=f"po_{tt}_{nb}") for nb in range(NOB)]
            for p4 in range(DQN):
                for nb in range(NOB):
                    nc.tensor.matmul(
                        pos[nb][:],
                        m(atp[:, p4, 128 * mm_:128 * (mm_ + 1)]),
                        m(wo_sb[:, p4, nb * NOUT:(nb + 1) * NOUT]),
                        start=(p4 == 0), stop=(p4 == DQN - 1))
            for nb in range(NOB):
                osb = stream.tile([128, NOUT], F32, tag="osb", bufs=6,
                                  name=f"ob_{tt}_{nb}")
                nc.vector.tensor_copy(osb[:], pos[nb][:])
                nc.sync.dma_start(
                    out[128 * tt:128 * (tt + 1),
                        nb * NOUT:(nb + 1) * NOUT], osb[:])

        for kt in range(DK):
            nc.gpsimd.dma_start(wv_sb[:, kt, :],
                                m(wv[128 * kt:128 * (kt + 1), :]))
        # bv broadcast [1,DQ] -> [128,DQ] via a K=1 matmul (emitted here so
        # it is not the head of the PE queue at startup)
        ones128f = const.tile([1, 128], F32)
        nc.vector.memset(ones128f[:], 1.0)
        ones128r = const.tile([1, 128], F32R)
        nc.vector.tensor_copy(ones128r[:], ones128f[:])
        bv_row = const.tile([1, DQ], F32R)
        nc.vector.tensor_copy(bv_row[:], bv_rowf[:])
        bv_bc = const.tile([128, DQ], F32)
        bv_ps = psum.tile([128, DQ], F32, tag="v", bufs=2)
        nc.tensor.matmul(bv_ps[:], r(ones128r[:]), r(bv_row[:]),
                         start=True, stop=True)
        nc.scalar.copy(bv_bc[:], bv_ps[:])
        for tt in range(SQ // 128):
            emit_v_tile(tt)
        for p4 in range(DQN):
            nc.gpsimd.dma_start(wo_sb[:, p4, :],
                                m(wo[128 * p4:128 * (p4 + 1), :]))
        # ones columns of v_aug: no dependency on the v-tile feat writes
        # (disjoint regions), first read is the first PV in phase B
        nc.vector.memset(v_aug[:, :, :, :, 0:DH], 1.0)

        for i in range(NSB):
            # ===== phase B: attention for superblock i =====================
            ND = SQ // 128
            NJ = ND * (i + 1)
            attnT = stream.tile([128, DQN, SQ], mm_dt, tag="attnT", bufs=2,
                                name=f"at_{i}")
            attn_tiles[i] = attnT
            pending = [None]  # deferred normalize of prev hp
            for hp in range(DQN):
                pva = psum.tile([128, SQ], F32, tag="v", bufs=2,
                                name=f"pv_{i}_{hp}_0")
                pvb = psum.tile([128, SQ], F32, tag="v", bufs=2,
                                name=f"pv_{i}_{hp}_1")
                pvs = (pva, pvb)
                pend = None
                def emit_pv(pend):
                    pprbs, pf0, pj = pend
                    for hh in range(2):
                        nc.tensor.matmul(
                            pvs[hh][:, pf0:],
                            m(v_aug[:, pj, hh, hp, :]),
                            m(pprbs[:, SQ * hh + pf0:SQ * (hh + 1)]),
                            start=(pj == 0), stop=(pj == NJ - 1))

                for j in range(NJ):
                    jj = j - ND * i
                    f0 = max(0, 128 * jj)
                    sc = psum.tile([128, 2 * SQ], F32, tag="qk",
                                   bufs=qk_psum_bufs, name=f"sc_{i}_{hp}_{j}")
                    for hh, qsrc in ((0, qT0), (1, qT1)):
                        nc.tensor.matmul(
                            sc[:, SQ * hh + f0:SQ * (hh + 1)],
                            m(kT[:, hp, 128 * j:128 * (j + 1)]),
                            m(qsrc[:, hp, i * SQ + f0:(i + 1) * SQ]),
                            start=True, stop=True)
                    if j == 1:
                        if pending[0] is not None:
                            pending[0]()
                            pending[0] = None
                        # PE filler while exp(j=0) runs: v proj for the next
                        # superblock's token tile hp (sb0 emits these at its
                        # hp boundaries next to the interleaved qk chains)
                        if 0 < i < NSB - 1:
                            emit_v_tile(ND * (i + 1) + hp)
                        # PE filler: output projection of the previous
                        # superblock.  For hp=0 its last normalize was only
                        # just emitted, so defer to j==3 for slack.
                        if i >= 1 and hp >= 1:
                            emit_out_tile(ND * (i - 1) + hp)
                    if j == 3 and i >= 1 and hp == 0:
                        emit_out_tile(ND * (i - 1))
                    probs = stream.tile([128, 2 * SQ], mm_dt, tag="probs",
                                        bufs=probs_bufs,
                                        name=f"pr_{i}_{hp}_{j}")
                    if f0 == 0:
                        nc.scalar.activation(
                            probs[:], sc[:],
                            mybir.ActivationFunctionType.Exp)
                    else:
                        nc.scalar.activation(
                            probs[:].rearrange("p (h c) -> p h c", h=2)[:, :, f0:],
                            sc[:].rearrange("p (h c) -> p h c", h=2)[:, :, f0:],
                            mybir.ActivationFunctionType.Exp)
                    if jj >= 0:
                        for hh in range(2):
                            nc.vector.tensor_tensor(
                                probs[:, SQ * hh + f0:SQ * hh + f0 + 128],
                                probs[:, SQ * hh + f0:SQ * hh + f0 + 128],
                                tri01[:], op=mybir.AluOpType.mult)
                    if pend is not None:
                        emit_pv(pend)
                    pend = (probs, f0, j)
                emit_pv(pend)

                def make_norm(pvs=pvs, hp=hp, at=attnT, ii=i):
                    def emit():
                        # sumexp rows 0:64 (recip works only at base
                        # partition 0), attn rows 64:128.  Reciprocal is
                        # DMA-shifted up (DVE lanes are partition-locked);
                        # hh=1 attn writes attnT rows 64:128 directly, hh=0
                        # goes through a staging tile + partition-shift DMA.
                        # Both recips first so the two shift DMAs overlap.
                        recs = []
                        for hh in range(2):
                            rec = stream.tile([128, SQ], F32, tag="rec",
                                              bufs=6, name=f"rc_{ii}_{hp}_{hh}")
                            nc.vector.reciprocal_approx_fast(
                                rec[0:64, :], pvs[hh][0:64, :])
                            recs.append(rec)
                        for rec in recs:
                            nc.sync.dma_start(rec[64:128, :], rec[0:64, :])
                        # longer chain (stage + second DMA) first
                        stage = stream.tile([128, SQ], mm_dt,
                                            tag="stage", bufs=2,
                                            name=f"st_{ii}_{hp}")
                        nc.vector.tensor_tensor(
                            stage[64:128, :],
                            pvs[0][64:128, :], recs[0][64:128, :],
                            op=mybir.AluOpType.mult)
                        nc.sync.dma_start(at[0:64, hp, :],
                                          stage[64:128, :])
                        nc.vector.tensor_tensor(
                            at[64:128, hp, :],
                            pvs[1][64:128, :], recs[1][64:128, :],
                            op=mybir.AluOpType.mult)
                    return emit

                pending[0] = make_norm()

                if i == 0:
                    # interleave the remaining qk projection chains at sb0's
                    # hp boundaries: the next hp's scores depend on them, so
                    # the PE stays dense instead of idling on exp latency.
                    # The v tile fills the chain's psum-drain latency.
                    if hp < DQN - 1:
                        emit_qk_chain(hp + 1)
                        emit_qk_chain(DQN + hp + 1)
                    emit_v_tile(ND + hp)

            if i == NSB - 1:
                # ===== phase C tail: last superblock's output projection.
                # Two tiles accumulate p4=0..2 in spare qk-tag psum BEFORE
                # the final normalize lands (they only read attnT hp 0..2),
                # overlapping the PE with the normalize chain.
                tail = []
                for k in range(ND):
                    tt = (NSB - 1) * ND + k
                    if k < 2:
                        tp = psum.tile([128, 2 * NOUT], F32, tag="qk",
                                       bufs=qk_psum_bufs, name=f"tpo_{tt}")
                        pos = [tp[:, 0:NOUT], tp[:, NOUT:2 * NOUT]]
                    else:
                        pos = [psum.tile([128, NOUT], F32,
                                         tag="v" if k == 2 else "out", bufs=2,
                                         name=f"tpo_{tt}_{nb}")[:]
                               for nb in range(NOB)]
                    tail.append((tt, pos))
                for tt, pos in tail[:2]:
                    mm_ = tt % ND
                    for p4 in range(DQN - 1):
                        for nb in range(NOB):
                            nc.tensor.matmul(
                                pos[nb],
                                m(attnT[:, p4, 128 * mm_:128 * (mm_ + 1)]),
                                m(wo_sb[:, p4, nb * NOUT:(nb + 1) * NOUT]),
                                start=(p4 == 0), stop=False)

            if pending[0] is not None:
                pending[0]()
                pending[0] = None

            if i == NSB - 1:
                for k, (tt, pos) in enumerate(tail):
                    mm_ = tt % ND
                    p4s = range(DQN - 1, DQN) if k < 2 else range(DQN)
                    for p4 in p4s:
                        for nb in range(NOB):
                            nc.tensor.matmul(
                                pos[nb],
                                m(attnT[:, p4, 128 * mm_:128 * (mm_ + 1)]),
                                m(wo_sb[:, p4, nb * NOUT:(nb + 1) * NOUT]),
                                start=(p4 == 0 and k >= 2),
                                stop=(p4 == DQN - 1))
                    for nb in range(NOB):
                        osb = stream.tile([128, NOUT], F32, tag="osb", bufs=6,
                                          name=f"ob_{tt}_{nb}")
                        nc.vector.tensor_copy(osb[:], pos[nb])
                        nc.sync.dma_start(
                            out[128 * tt:128 * (tt + 1),
                                nb * NOUT:(nb + 1) * NOUT], osb[:])

    nc.compile()
    return nc

B, S, D, H = 4, 2048, 1024, 16
N_CORES = 8

_CACHED = {}


def _make_core_inputs(x, Wq, bq, Wk, bk, Wv, bv, Wo):
    DQ = D // 2

    def cast(a):
        return np.ascontiguousarray(a).astype(ml_dtypes.bfloat16)

    xTs = [cast(x[b].T) for b in range(B)]
    in_maps = []
    for c in range(N_CORES):
        b, hf = c // 2, c % 2
        sl = slice(hf * DQ, (hf + 1) * DQ)
        in_maps.append({
            "xT": xTs[b],
            "wqk": cast(np.concatenate([Wq[:, sl], Wk[:, sl]], axis=1)),
            "wv": cast(Wv[:, sl]),
            "wo": cast(Wo[sl, :]),
            "bqk": np.ascontiguousarray(
                np.concatenate([0.125 * bq[sl], bk[sl]])).astype(np.float32),
            "bv": np.ascontiguousarray(bv[sl]).astype(np.float32),
        })
    return in_maps


def kernel(x, Wq, bq, Wk, bk, Wv, bv, Wo, bo):
    import tempfile
    from concourse import bass_utils

    x = np.asarray(x, dtype=np.float32)
    Wq = np.asarray(Wq, dtype=np.float32)
    bq = np.asarray(bq, dtype=np.float32)
    Wk = np.asarray(Wk, dtype=np.float32)
    bk = np.asarray(bk, dtype=np.float32)
    Wv = np.asarray(Wv, dtype=np.float32)
    bv = np.asarray(bv, dtype=np.float32)
    Wo = np.asarray(Wo, dtype=np.float32)
    bo = np.asarray(bo, dtype=np.float32)

    if "nc" not in _CACHED:
        _CACHED["nc"] = build_core_program(S=S, D=D, HC=H // 2)
    nc = _CACHED["nc"]

    in_maps = _make_core_inputs(x, Wq, bq, Wk, bk, Wv, bv, Wo)
    res = bass_utils.run_bass_kernel_spmd(
        nc, in_maps, core_ids=list(range(N_CORES)),
        tmpdir=tempfile.mkdtemp(prefix="bass_attn_"))

    out = np.empty((B, S, D), dtype=np.float32)
    for b in range(B):
        out[b] = res.results[2 * b]["out"] + res.results[2 * b + 1]["out"] + bo
    return out
